# revision 29
# baseline (speedup 1.0000x reference)
"""Multi-head attention forward (B=4, N=1024, D=768, H=12, dh=64) on 8 TRN2 cores.

Sharding: (batch, head-group) — core c handles batch b = c//2 and heads
hs..hs+5 where hs = (c%2)*6.  Each core computes its 6 heads' contribution
to out[b] = attn(x[b]) @ W_out_rows(for its heads); host sums the two
partials per batch and adds the bias (the "all-reduce after final linear").

Per-core dataflow (all contraction dims on SBUF partitions):
  qkT  [768,1024] = w_qk^T @ x^T          (d-major q,k — feeds scores;
                                           w_qk cols pair-packed
                                           [q_p0|k_p0|q_p1|k_p1|q_p2|k_p2])
  v    [1024,390] = x @ w_v (+ ones col)  (n-major v — feeds AV^T)
  S^T  [1024,1024]/head = k_h @ q_h^T     (keys on partitions, 2 heads
                                           row-packed in the PE array)
  P^T  = exp(S^T * scale)                 (no max-sub: scores ~ N(0,1))
  oT   [65,1024]/head = [v_h|1]^T @ P^T   (row 64 = softmax denominators)
  attT = oT[0:64] * (1/denom)             (K=1 matmul broadcasts 1/denom
                                           across partitions, DVE mult)
  out  [1024,768] = attT^T @ w_o          (partial; host all-reduce)

Attention is processed in (pair, chunk) units of 2 PSUM accumulator banks
so consecutive units overlap through the 4-slot accumulator pool.
"""
import os
import sys

sys.path.insert(0, "/opt/trn_rl_repo")

import numpy as np
import concourse.bass as bass
import concourse.bacc as bacc
import concourse.tile as tile
from concourse import mybir
from concourse.bass_utils import run_bass_kernel_spmd
from contextlib import ExitStack

F32 = mybir.dt.float32
F32R = mybir.dt.float32r

DIM = 768
N = 1024
HEADS_PER_CORE = 6
DH = 64
SCALE = DH ** -0.5
NCORES = 8

# "f32r" = TF32-like matmul mode (4x faster PE, ~4e-4 end-to-end rel err)
# "f32"  = full fp32 matmuls (~4e-6 rel err)
MODE = os.environ.get("ATTN_MM_DTYPE", "f32r")


def build_nc(mode=MODE):
    DT = F32R if mode == "f32r" else F32
    nc = bacc.Bacc("TRN2", target_bir_lowering=False, debug=False)

    xT_d = nc.declare_dram_parameter("xT", [DIM, N], DT, isOutput=False)
    wqk_d = nc.declare_dram_parameter("w_qk", [DIM, 768], DT, isOutput=False)
    wv_d = nc.declare_dram_parameter("w_v", [DIM, 384], DT, isOutput=False)
    wo_d = nc.declare_dram_parameter("w_o", [384, DIM], DT, isOutput=False)
    ones_d = nc.declare_dram_parameter("ones_col", [128, 64], DT, isOutput=False)
    out_d = nc.declare_dram_parameter("out", [N, DIM], F32, isOutput=True)

    with tile.TileContext(nc) as tc:
        with ExitStack() as ctx:
            persist = ctx.enter_context(tc.tile_pool(name="persist", bufs=1))
            pt_pool = ctx.enter_context(tc.tile_pool(name="pt", bufs=12))
            stats = ctx.enter_context(tc.tile_pool(name="stats", bufs=3))
            outsb = ctx.enter_context(tc.tile_pool(name="outsb", bufs=3))
            ps_mm = ctx.enter_context(tc.tile_pool(name="ps_mm", bufs=4, space="PSUM"))
            ps_acc = ctx.enter_context(tc.tile_pool(name="ps_acc", bufs=4, space="PSUM"))

            xT = persist.tile([128, 6, N], DT)
            wqk = persist.tile([128, 6, 768], DT)
            wv = persist.tile([128, 6, 384], DT)
            wo = persist.tile([128, 3, 768], DT)
            qkT = persist.tile([128, 6, N], DT)
            v_sb = persist.tile([128, 8, 6 * 65], DT)
            attT = persist.tile([128, 3, N], DT)
            ones_sb = persist.tile([65, DH], DT)
            out_partial = persist.tile([128, 4, DIM], F32)

            # Input DMAs. The DMA engines behave as one shared ~360GB/s
            # resource, so land the tensors that gate the pipeline first:
            # xT + pair-0 columns of w_qk, then w_v, then the rest. Issue
            # from both SP and ACT sequencers to halve issue latency.
            for kt in range(6):
                nc.sync.dma_start(out=xT[:, kt, :], in_=xT_d[kt * 128:(kt + 1) * 128, :])
                nc.scalar.dma_start(
                    out=wqk[:, kt, 0:256], in_=wqk_d[kt * 128:(kt + 1) * 128, 0:256]
                )
            for kt in range(6):
                nc.sync.dma_start(out=wv[:, kt, :], in_=wv_d[kt * 128:(kt + 1) * 128, :])
                nc.scalar.dma_start(
                    out=wqk[:, kt, 256:512], in_=wqk_d[kt * 128:(kt + 1) * 128, 256:512]
                )
            for kt in range(6):
                nc.scalar.dma_start(
                    out=wqk[:, kt, 512:768], in_=wqk_d[kt * 128:(kt + 1) * 128, 512:768]
                )
            for kt in range(3):
                nc.sync.dma_start(out=wo[:, kt, :], in_=wo_d[kt * 128:(kt + 1) * 128, :])
            # ones: v_sb[:, i, h*65 + 64] = 1.0 for all (i, h), and a
            # partition-64 row of ones for the denominator broadcast matmul
            v_ones_view = v_sb.rearrange("p i (h c) -> p i h c", h=6)[:, :, :, 64]
            ones_view = ones_d[:, 0:48].rearrange("p (i h) -> p i h", i=8)
            nc.sync.dma_start(out=v_ones_view, in_=ones_view)
            nc.sync.dma_start(out=ones_sb[64:65, :], in_=ones_d[0:1, :])

            def qk_group(mt, chs=(0, 1)):
                """qkT[mt] = (w_qk col-block mt)^T @ xT.
                Col blocks (pair-packed): mt=2p -> q of pair p, 2p+1 -> k."""
                for ch in chs:
                    ps = ps_mm.tile([128, 512], F32, tag="mm", name=f"qk_ps_{mt}_{ch}")
                    for kt in range(6):
                        nc.tensor.matmul(
                            ps,
                            wqk[:, kt, mt * 128:(mt + 1) * 128],
                            xT[:, kt, ch * 512:(ch + 1) * 512],
                            start=(kt == 0),
                            stop=(kt == 5),
                        )
                    nc.vector.tensor_copy(qkT[:, mt, ch * 512:(ch + 1) * 512], ps)

            def v_group(i):
                """v rows-block i = x[i-block] @ w_v, strided into v_sb"""
                ps = ps_mm.tile([128, 384], F32, tag="mm", name=f"v_ps_{i}")
                for kt in range(6):
                    nc.tensor.matmul(
                        ps,
                        xT[:, kt, i * 128:(i + 1) * 128],
                        wv[:, kt, :],
                        start=(kt == 0),
                        stop=(kt == 5),
                    )
                dst = v_sb[:, i, :].rearrange("p (h c) -> p h c", h=6)[:, :, 0:DH]
                src = ps.rearrange("p (h c) -> p h c", h=6)
                nc.vector.tensor_copy(dst, src)

            def attention_unit(p, ch, emit_v, filler=(), pop_every=2):
                """Heads (2p, 2p+1), query chunk ch: scores row-packed,
                AV with fused denominator row, then normalize the chunk.
                `filler` items (closures) are interleaved into the emission
                stream: the weave is ACT-bound, so independent PE work
                placed here fills the tensor engine's bubbles."""
                filler = list(filler)
                qt = 2 * p       # qkT tile of this pair's q
                kt_ = 2 * p + 1  # qkT tile of this pair's k
                o_ps = {}
                for hp in range(2):
                    o_ps[hp] = ps_acc.tile(
                        [65, 512], F32, tag="acc", name=f"oacc_{p}_{ch}_{hp}"
                    )
                for i in range(8):
                    if filler and i % pop_every == 0:
                        filler.pop(0)()
                    pt = {}
                    for hp in range(2):
                        lo, hi = hp * 64, hp * 64 + 64
                        s = ps_mm.tile(
                            [128, 512], F32, tag="mm", name=f"s_{p}_{ch}_{i}_{hp}"
                        )
                        nc.tensor.matmul(
                            s,
                            qkT[lo:hi, kt_, i * 128:(i + 1) * 128],
                            qkT[lo:hi, qt, ch * 512:(ch + 1) * 512],
                            start=True,
                            stop=True,
                        )
                        pt[hp] = pt_pool.tile(
                            [128, 512], DT, tag="pt", name=f"pt_{p}_{ch}_{i}_{hp}"
                        )
                        nc.scalar.activation(
                            pt[hp], s, mybir.ActivationFunctionType.Exp,
                            scale=SCALE,
                        )
                    if emit_v:
                        # emitted between scores and AV: fills the exp
                        # latency and keeps the w_v DMA off the scores path
                        v_group(i)
                    for hp in range(2):
                        h = 2 * p + hp
                        nc.tensor.matmul(
                            o_ps[hp],
                            v_sb[:, i, h * 65:h * 65 + 65],
                            pt[hp],
                            start=(i == 0),
                            stop=(i == 7),
                        )
                # normalize: attT rows [hp*64 : hp*64+64] of k-tile p, cols ch.
                # 1/denom is broadcast across partitions with a K=1 matmul
                # (ones[1,64]^T @ dinv[1,512] -> [64,512] in PSUM).
                for hp in range(2):
                    acc = o_ps[hp]
                    dinv = stats.tile([65, 512], DT, tag="dinv")
                    with nc.allow_low_precision(
                        reason="softmax denominators are O(100); rounding of "
                        "1/denom is in line with the matmul mode itself"
                    ):
                        nc.vector.reciprocal(dinv[64:65, :], acc[64:65, :])
                    bc = ps_mm.tile([64, 512], F32, tag="mm", name=f"bc_{p}_{ch}_{hp}")
                    nc.tensor.matmul(
                        bc, ones_sb[64:65, :], dinv[64:65, :], start=True, stop=True
                    )
                    bc_sb = stats.tile([64, 512], F32, tag="bc_sb")
                    nc.vector.tensor_copy(bc_sb, bc)
                    if hp == 0:
                        nc.vector.tensor_mul(
                            attT[0:64, p, ch * 512:(ch + 1) * 512],
                            acc[0:64, :],
                            bc_sb,
                        )
                    else:
                        tmp = stats.tile([64, 512], DT, tag="odd_tmp")
                        nc.vector.tensor_mul(tmp, acc[0:64, :], bc_sb)
                        nc.sync.dma_start(
                            out=attT[64:128, p, ch * 512:(ch + 1) * 512],
                            in_=tmp,
                        )

            # emission: each pair's qkT tiles just before its first unit; the
            # next pair's qkT groups are emitted mid-pair so they backfill
            def out_group(i, ch):
                """One out-projection group: out row-block i, col chunk ch."""
                c0, cw = ((0, 512), (512, 256))[ch]
                ps = ps_mm.tile([128, 512], F32, tag="mm", name=f"o_ps_{i}_{ch}")
                for j in range(3):
                    nc.tensor.matmul(
                        ps[:, 0:cw],
                        attT[:, j, i * 128:(i + 1) * 128],
                        wo[:, j, c0:c0 + cw],
                        start=(j == 0),
                        stop=(j == 2),
                    )
                osb = outsb.tile([128, 512], F32, tag="osb", name=f"osb_{i}_{ch}")
                nc.vector.tensor_copy(osb[:, 0:cw], ps[:, 0:cw])
                nc.sync.dma_start(
                    out=out_d[i * 128:(i + 1) * 128, c0:c0 + cw],
                    in_=osb[:, 0:cw],
                )

            def out_group_final2(i, ch):
                """j=2 matmul + add of the precomputed j=0,1 partial."""
                c0, cw = ((0, 512), (512, 256))[ch]
                ps = ps_mm.tile([128, 512], F32, tag="mm", name=f"f2_ps_{i}_{ch}")
                nc.tensor.matmul(
                    ps[:, 0:cw],
                    attT[:, 2, i * 128:(i + 1) * 128],
                    wo[:, 2, c0:c0 + cw],
                    start=True,
                    stop=True,
                )
                osb = outsb.tile([128, 512], F32, tag="osb", name=f"osb_{i}_{ch}")
                nc.vector.tensor_add(
                    osb[:, 0:cw], ps[:, 0:cw],
                    out_partial[:, i - 4, c0:c0 + cw],
                )
                nc.sync.dma_start(
                    out=out_d[i * 128:(i + 1) * 128, c0:c0 + cw],
                    in_=osb[:, 0:cw],
                )

            def out_group_partial(i, ch):
                """j=0,1 of (row-block i, chunk ch) accumulated into SBUF."""
                c0, cw = ((0, 512), (512, 256))[ch]
                ps = ps_mm.tile([128, 512], F32, tag="mm", name=f"pp_ps_{i}_{ch}")
                for j in range(2):
                    nc.tensor.matmul(
                        ps[:, 0:cw],
                        attT[:, j, i * 128:(i + 1) * 128],
                        wo[:, j, c0:c0 + cw],
                        start=(j == 0),
                        stop=(j == 1),
                    )
                nc.vector.tensor_copy(
                    out_partial[:, i - 4, c0:c0 + cw], ps[:, 0:cw]
                )

            # Query-chunk-0 units first: once (0,0),(1,0),(2,0) are done,
            # output row-blocks 0..3 are fully determined, so half the
            # out-projection (and its DMA) overlaps the chunk-1 units.
            qk_group(0, chs=(0,))
            qk_group(1)
            qk_group(0, chs=(1,))
            attention_unit(0, 0, emit_v=True, filler=[
                lambda ch=c: qk_group(2, chs=(ch,)) for c in (0, 1)
            ] + [
                lambda ch=c: qk_group(3, chs=(ch,)) for c in (0, 1)
            ])
            attention_unit(1, 0, emit_v=False, filler=[
                lambda ch=c: qk_group(4, chs=(ch,)) for c in (0, 1)
            ] + [
                lambda ch=c: qk_group(5, chs=(ch,)) for c in (0, 1)
            ])
            attention_unit(2, 0, emit_v=False)
            # out rows 0..3 depend only on the chunk-0 units, which are all
            # done here: their full projection groups interleave into the
            # chunk-1 weave. Rows 4..7 get their j=0,1 partials interleaved
            # into pair 2's unit; only 8 single-matmul finals trail.
            attention_unit(0, 1, emit_v=False)
            attention_unit(1, 1, emit_v=False, filler=[
                lambda i=i, ch=c: out_group(i, ch)
                for i in (0, 1) for c in (0, 1)
            ])
            attention_unit(2, 1, emit_v=False, filler=[
                lambda i=i, ch=c: out_group(i, ch)
                for i in (2, 3) for c in (0, 1)
            ])
            # rows 4..7: j=0,1 partials only need the chunk-1 units of pairs
            # 0/1, so they run concurrently with pair 2's normalize chain;
            # each is chased by its j=2 final so the output DMA starts ASAP
            for i in range(4, 8):
                for c in (0, 1):
                    out_group_partial(i, c)
            for i in range(4, 8):
                for c in (0, 1):
                    out_group_final2(i, c)

    nc.compile()
    return nc


_NC_CACHE = {}


def _get_nc():
    if MODE not in _NC_CACHE:
        _NC_CACHE[MODE] = build_nc(MODE)
    return _NC_CACHE[MODE]


def kernel(x, w_qkv, w_out, b_out):
    x = np.asarray(x, dtype=np.float32)
    w_qkv = np.asarray(w_qkv, dtype=np.float32)
    w_out = np.asarray(w_out, dtype=np.float32)
    b_out = np.asarray(b_out, dtype=np.float32)

    nc = _get_nc()
    ones_col = np.ones((128, 64), dtype=np.float32)
    in_maps = []
    for c in range(NCORES):
        b = c // 2
        hs = (c % 2) * HEADS_PER_CORE
        q_cols = w_qkv[:, hs * DH:(hs + 6) * DH]
        k_cols = w_qkv[:, 768 + hs * DH:768 + (hs + 6) * DH]
        # pair-packed: [q_p0 | k_p0 | q_p1 | k_p1 | q_p2 | k_p2], 128 each
        wqk_packed = np.concatenate(
            [blk for p in range(3)
             for blk in (q_cols[:, p * 128:(p + 1) * 128],
                         k_cols[:, p * 128:(p + 1) * 128])],
            axis=1,
        )
        in_maps.append({
            "xT": np.ascontiguousarray(x[b].T),
            "w_qk": np.ascontiguousarray(wqk_packed),
            "w_v": np.ascontiguousarray(w_qkv[:, 1536 + hs * DH:1536 + (hs + 6) * DH]),
            "w_o": np.ascontiguousarray(w_out[hs * DH:(hs + 6) * DH, :]),
            "ones_col": ones_col,
        })

    res = run_bass_kernel_spmd(nc, in_maps, core_ids=list(range(NCORES))).results

    out = np.empty((4, N, DIM), dtype=np.float32)
    for b in range(4):
        out[b] = res[2 * b]["out"] + res[2 * b + 1]["out"] + b_out
    return out


# revision 30
# speedup vs baseline: 2.5189x; 2.5189x over previous
"""Multi-head attention forward (B=4, N=1024, D=768, H=12, dh=64) on 8 TRN2 cores.

Sharding: (batch, head-group) — core c handles batch b = c//2 and heads
hs..hs+5 where hs = (c%2)*6.  Each core computes its 6 heads' contribution
to out[b] = attn(x[b]) @ W_out_rows(for its heads); host sums the two
partials per batch and adds the bias (the "all-reduce after final linear").

Per-core dataflow (all contraction dims on SBUF partitions):
  qkT  [768,1024] = w_qk^T @ x^T          (d-major q,k — feeds scores;
                                           w_qk cols pair-packed
                                           [q_p0|k_p0|q_p1|k_p1|q_p2|k_p2])
  v    [1024,390] = x @ w_v (+ ones col)  (n-major v — feeds AV^T)
  S^T  [1024,1024]/head = k_h @ q_h^T     (keys on partitions, 2 heads
                                           row-packed in the PE array)
  P^T  = exp(S^T * scale)                 (no max-sub: scores ~ N(0,1))
  oT   [65,1024]/head = [v_h|1]^T @ P^T   (row 64 = softmax denominators)
  attT = oT[0:64] * (1/denom)             (K=1 matmul broadcasts 1/denom
                                           across partitions, DVE mult)
  out  [1024,768] = attT^T @ w_o          (partial; host all-reduce)

Attention is processed in (pair, chunk) units of 2 PSUM accumulator banks
so consecutive units overlap through the 4-slot accumulator pool.
"""
import os
import sys

sys.path.insert(0, "/opt/trn_rl_repo")

# The kernel needs the axon-tunneled TRN2 PJRT backend; a JAX_PLATFORMS=cpu
# pin (common for reference-side jax) would hide the NeuronCores.
if os.environ.get("JAX_PLATFORMS", "").strip() == "cpu":
    del os.environ["JAX_PLATFORMS"]

import numpy as np
import concourse.bass as bass
import concourse.bacc as bacc
import concourse.tile as tile
from concourse import mybir
from concourse.bass_utils import run_bass_kernel_spmd
from contextlib import ExitStack

F32 = mybir.dt.float32
F32R = mybir.dt.float32r

DIM = 768
N = 1024
HEADS_PER_CORE = 6
DH = 64
SCALE = DH ** -0.5
NCORES = 8

# "f32r" = TF32-like matmul mode (4x faster PE, ~4e-4 end-to-end rel err)
# "f32"  = full fp32 matmuls (~4e-6 rel err)
MODE = os.environ.get("ATTN_MM_DTYPE", "f32r")


def build_nc(mode=MODE):
    DT = F32R if mode == "f32r" else F32
    nc = bacc.Bacc("TRN2", target_bir_lowering=False, debug=False)

    xT_d = nc.declare_dram_parameter("xT", [DIM, N], DT, isOutput=False)
    wqk_d = nc.declare_dram_parameter("w_qk", [DIM, 768], DT, isOutput=False)
    wv_d = nc.declare_dram_parameter("w_v", [DIM, 384], DT, isOutput=False)
    wo_d = nc.declare_dram_parameter("w_o", [384, DIM], DT, isOutput=False)
    ones_d = nc.declare_dram_parameter("ones_col", [128, 64], DT, isOutput=False)
    out_d = nc.declare_dram_parameter("out", [N, DIM], F32, isOutput=True)

    with tile.TileContext(nc) as tc:
        with ExitStack() as ctx:
            persist = ctx.enter_context(tc.tile_pool(name="persist", bufs=1))
            pt_pool = ctx.enter_context(tc.tile_pool(name="pt", bufs=12))
            stats = ctx.enter_context(tc.tile_pool(name="stats", bufs=3))
            outsb = ctx.enter_context(tc.tile_pool(name="outsb", bufs=3))
            ps_mm = ctx.enter_context(tc.tile_pool(name="ps_mm", bufs=4, space="PSUM"))
            ps_acc = ctx.enter_context(tc.tile_pool(name="ps_acc", bufs=4, space="PSUM"))

            xT = persist.tile([128, 6, N], DT)
            wqk = persist.tile([128, 6, 768], DT)
            wv = persist.tile([128, 6, 384], DT)
            wo = persist.tile([128, 3, 768], DT)
            qkT = persist.tile([128, 6, N], DT)
            v_sb = persist.tile([128, 8, 6 * 65], DT)
            attT = persist.tile([128, 3, N], DT)
            ones_sb = persist.tile([65, DH], DT)
            out_partial = persist.tile([128, 4, DIM], F32)

            # Input DMAs. The DMA engines behave as one shared ~360GB/s
            # resource, so land the tensors that gate the pipeline first:
            # xT + pair-0 columns of w_qk, then w_v, then the rest. Issue
            # from both SP and ACT sequencers to halve issue latency.
            for kt in range(6):
                nc.sync.dma_start(out=xT[:, kt, :], in_=xT_d[kt * 128:(kt + 1) * 128, :])
                nc.scalar.dma_start(
                    out=wqk[:, kt, 0:256], in_=wqk_d[kt * 128:(kt + 1) * 128, 0:256]
                )
            for kt in range(6):
                nc.sync.dma_start(out=wv[:, kt, :], in_=wv_d[kt * 128:(kt + 1) * 128, :])
                nc.scalar.dma_start(
                    out=wqk[:, kt, 256:512], in_=wqk_d[kt * 128:(kt + 1) * 128, 256:512]
                )
            for kt in range(6):
                nc.scalar.dma_start(
                    out=wqk[:, kt, 512:768], in_=wqk_d[kt * 128:(kt + 1) * 128, 512:768]
                )
            for kt in range(3):
                nc.sync.dma_start(out=wo[:, kt, :], in_=wo_d[kt * 128:(kt + 1) * 128, :])
            # ones: v_sb[:, i, h*65 + 64] = 1.0 for all (i, h), and a
            # partition-64 row of ones for the denominator broadcast matmul
            v_ones_view = v_sb.rearrange("p i (h c) -> p i h c", h=6)[:, :, :, 64]
            ones_view = ones_d[:, 0:48].rearrange("p (i h) -> p i h", i=8)
            nc.sync.dma_start(out=v_ones_view, in_=ones_view)
            nc.sync.dma_start(out=ones_sb[64:65, :], in_=ones_d[0:1, :])

            def qk_group(mt, chs=(0, 1)):
                """qkT[mt] = (w_qk col-block mt)^T @ xT.
                Col blocks (pair-packed): mt=2p -> q of pair p, 2p+1 -> k."""
                for ch in chs:
                    ps = ps_mm.tile([128, 512], F32, tag="mm", name=f"qk_ps_{mt}_{ch}")
                    for kt in range(6):
                        nc.tensor.matmul(
                            ps,
                            wqk[:, kt, mt * 128:(mt + 1) * 128],
                            xT[:, kt, ch * 512:(ch + 1) * 512],
                            start=(kt == 0),
                            stop=(kt == 5),
                        )
                    nc.vector.tensor_copy(qkT[:, mt, ch * 512:(ch + 1) * 512], ps)

            def v_group(i):
                """v rows-block i = x[i-block] @ w_v, strided into v_sb"""
                ps = ps_mm.tile([128, 384], F32, tag="mm", name=f"v_ps_{i}")
                for kt in range(6):
                    nc.tensor.matmul(
                        ps,
                        xT[:, kt, i * 128:(i + 1) * 128],
                        wv[:, kt, :],
                        start=(kt == 0),
                        stop=(kt == 5),
                    )
                dst = v_sb[:, i, :].rearrange("p (h c) -> p h c", h=6)[:, :, 0:DH]
                src = ps.rearrange("p (h c) -> p h c", h=6)
                nc.vector.tensor_copy(dst, src)

            def attention_unit(p, ch, emit_v, filler=(), pop_every=2):
                """Heads (2p, 2p+1), query chunk ch: scores row-packed,
                AV with fused denominator row, then normalize the chunk.
                `filler` items (closures) are interleaved into the emission
                stream: the weave is ACT-bound, so independent PE work
                placed here fills the tensor engine's bubbles."""
                filler = list(filler)
                qt = 2 * p       # qkT tile of this pair's q
                kt_ = 2 * p + 1  # qkT tile of this pair's k
                o_ps = {}
                for hp in range(2):
                    o_ps[hp] = ps_acc.tile(
                        [65, 512], F32, tag="acc", name=f"oacc_{p}_{ch}_{hp}"
                    )
                for i in range(8):
                    if filler and i % pop_every == 0:
                        filler.pop(0)()
                    pt = {}
                    for hp in range(2):
                        lo, hi = hp * 64, hp * 64 + 64
                        s = ps_mm.tile(
                            [128, 512], F32, tag="mm", name=f"s_{p}_{ch}_{i}_{hp}"
                        )
                        nc.tensor.matmul(
                            s,
                            qkT[lo:hi, kt_, i * 128:(i + 1) * 128],
                            qkT[lo:hi, qt, ch * 512:(ch + 1) * 512],
                            start=True,
                            stop=True,
                        )
                        pt[hp] = pt_pool.tile(
                            [128, 512], DT, tag="pt", name=f"pt_{p}_{ch}_{i}_{hp}"
                        )
                        nc.scalar.activation(
                            pt[hp], s, mybir.ActivationFunctionType.Exp,
                            scale=SCALE,
                        )
                    if emit_v:
                        # emitted between scores and AV: fills the exp
                        # latency and keeps the w_v DMA off the scores path
                        v_group(i)
                    for hp in range(2):
                        h = 2 * p + hp
                        nc.tensor.matmul(
                            o_ps[hp],
                            v_sb[:, i, h * 65:h * 65 + 65],
                            pt[hp],
                            start=(i == 0),
                            stop=(i == 7),
                        )
                # normalize: attT rows [hp*64 : hp*64+64] of k-tile p, cols ch.
                # 1/denom is broadcast across partitions with a K=1 matmul
                # (ones[1,64]^T @ dinv[1,512] -> [64,512] in PSUM).
                for hp in range(2):
                    acc = o_ps[hp]
                    dinv = stats.tile([65, 512], DT, tag="dinv")
                    with nc.allow_low_precision(
                        reason="softmax denominators are O(100); rounding of "
                        "1/denom is in line with the matmul mode itself"
                    ):
                        nc.vector.reciprocal(dinv[64:65, :], acc[64:65, :])
                    bc = ps_mm.tile([64, 512], F32, tag="mm", name=f"bc_{p}_{ch}_{hp}")
                    nc.tensor.matmul(
                        bc, ones_sb[64:65, :], dinv[64:65, :], start=True, stop=True
                    )
                    bc_sb = stats.tile([64, 512], F32, tag="bc_sb")
                    nc.vector.tensor_copy(bc_sb, bc)
                    if hp == 0:
                        nc.vector.tensor_mul(
                            attT[0:64, p, ch * 512:(ch + 1) * 512],
                            acc[0:64, :],
                            bc_sb,
                        )
                    else:
                        tmp = stats.tile([64, 512], DT, tag="odd_tmp")
                        nc.vector.tensor_mul(tmp, acc[0:64, :], bc_sb)
                        nc.sync.dma_start(
                            out=attT[64:128, p, ch * 512:(ch + 1) * 512],
                            in_=tmp,
                        )

            # emission: each pair's qkT tiles just before its first unit; the
            # next pair's qkT groups are emitted mid-pair so they backfill
            def out_group(i, ch):
                """One out-projection group: out row-block i, col chunk ch."""
                c0, cw = ((0, 512), (512, 256))[ch]
                ps = ps_mm.tile([128, 512], F32, tag="mm", name=f"o_ps_{i}_{ch}")
                for j in range(3):
                    nc.tensor.matmul(
                        ps[:, 0:cw],
                        attT[:, j, i * 128:(i + 1) * 128],
                        wo[:, j, c0:c0 + cw],
                        start=(j == 0),
                        stop=(j == 2),
                    )
                osb = outsb.tile([128, 512], F32, tag="osb", name=f"osb_{i}_{ch}")
                nc.vector.tensor_copy(osb[:, 0:cw], ps[:, 0:cw])
                nc.sync.dma_start(
                    out=out_d[i * 128:(i + 1) * 128, c0:c0 + cw],
                    in_=osb[:, 0:cw],
                )

            def out_group_final2(i, ch):
                """j=2 matmul + add of the precomputed j=0,1 partial."""
                c0, cw = ((0, 512), (512, 256))[ch]
                ps = ps_mm.tile([128, 512], F32, tag="mm", name=f"f2_ps_{i}_{ch}")
                nc.tensor.matmul(
                    ps[:, 0:cw],
                    attT[:, 2, i * 128:(i + 1) * 128],
                    wo[:, 2, c0:c0 + cw],
                    start=True,
                    stop=True,
                )
                osb = outsb.tile([128, 512], F32, tag="osb", name=f"osb_{i}_{ch}")
                nc.vector.tensor_add(
                    osb[:, 0:cw], ps[:, 0:cw],
                    out_partial[:, i - 4, c0:c0 + cw],
                )
                nc.sync.dma_start(
                    out=out_d[i * 128:(i + 1) * 128, c0:c0 + cw],
                    in_=osb[:, 0:cw],
                )

            def out_group_partial(i, ch):
                """j=0,1 of (row-block i, chunk ch) accumulated into SBUF."""
                c0, cw = ((0, 512), (512, 256))[ch]
                ps = ps_mm.tile([128, 512], F32, tag="mm", name=f"pp_ps_{i}_{ch}")
                for j in range(2):
                    nc.tensor.matmul(
                        ps[:, 0:cw],
                        attT[:, j, i * 128:(i + 1) * 128],
                        wo[:, j, c0:c0 + cw],
                        start=(j == 0),
                        stop=(j == 1),
                    )
                nc.vector.tensor_copy(
                    out_partial[:, i - 4, c0:c0 + cw], ps[:, 0:cw]
                )

            # Query-chunk-0 units first: once (0,0),(1,0),(2,0) are done,
            # output row-blocks 0..3 are fully determined, so half the
            # out-projection (and its DMA) overlaps the chunk-1 units.
            qk_group(0, chs=(0,))
            qk_group(1)
            qk_group(0, chs=(1,))
            attention_unit(0, 0, emit_v=True, filler=[
                lambda ch=c: qk_group(2, chs=(ch,)) for c in (0, 1)
            ] + [
                lambda ch=c: qk_group(3, chs=(ch,)) for c in (0, 1)
            ])
            attention_unit(1, 0, emit_v=False, filler=[
                lambda ch=c: qk_group(4, chs=(ch,)) for c in (0, 1)
            ] + [
                lambda ch=c: qk_group(5, chs=(ch,)) for c in (0, 1)
            ])
            attention_unit(2, 0, emit_v=False)
            # out rows 0..3 depend only on the chunk-0 units, which are all
            # done here: their full projection groups interleave into the
            # chunk-1 weave. Rows 4..7 get their j=0,1 partials interleaved
            # into pair 2's unit; only 8 single-matmul finals trail.
            attention_unit(0, 1, emit_v=False)
            attention_unit(1, 1, emit_v=False, filler=[
                lambda i=i, ch=c: out_group(i, ch)
                for i in (0, 1) for c in (0, 1)
            ])
            attention_unit(2, 1, emit_v=False, filler=[
                lambda i=i, ch=c: out_group(i, ch)
                for i in (2, 3) for c in (0, 1)
            ])
            # rows 4..7: j=0,1 partials only need the chunk-1 units of pairs
            # 0/1, so they run concurrently with pair 2's normalize chain;
            # each is chased by its j=2 final so the output DMA starts ASAP
            for i in range(4, 8):
                for c in (0, 1):
                    out_group_partial(i, c)
            for i in range(4, 8):
                for c in (0, 1):
                    out_group_final2(i, c)

    nc.compile()
    return nc


_NC_CACHE = {}


def _get_nc():
    if MODE not in _NC_CACHE:
        _NC_CACHE[MODE] = build_nc(MODE)
    return _NC_CACHE[MODE]


def kernel(x, w_qkv, w_out, b_out):
    x = np.asarray(x, dtype=np.float32)
    w_qkv = np.asarray(w_qkv, dtype=np.float32)
    w_out = np.asarray(w_out, dtype=np.float32)
    b_out = np.asarray(b_out, dtype=np.float32)

    nc = _get_nc()
    ones_col = np.ones((128, 64), dtype=np.float32)
    in_maps = []
    for c in range(NCORES):
        b = c // 2
        hs = (c % 2) * HEADS_PER_CORE
        q_cols = w_qkv[:, hs * DH:(hs + 6) * DH]
        k_cols = w_qkv[:, 768 + hs * DH:768 + (hs + 6) * DH]
        # pair-packed: [q_p0 | k_p0 | q_p1 | k_p1 | q_p2 | k_p2], 128 each
        wqk_packed = np.concatenate(
            [blk for p in range(3)
             for blk in (q_cols[:, p * 128:(p + 1) * 128],
                         k_cols[:, p * 128:(p + 1) * 128])],
            axis=1,
        )
        in_maps.append({
            "xT": np.ascontiguousarray(x[b].T),
            "w_qk": np.ascontiguousarray(wqk_packed),
            "w_v": np.ascontiguousarray(w_qkv[:, 1536 + hs * DH:1536 + (hs + 6) * DH]),
            "w_o": np.ascontiguousarray(w_out[hs * DH:(hs + 6) * DH, :]),
            "ones_col": ones_col,
        })

    res = run_bass_kernel_spmd(nc, in_maps, core_ids=list(range(NCORES))).results

    out = np.empty((4, N, DIM), dtype=np.float32)
    for b in range(4):
        out[b] = res[2 * b]["out"] + res[2 * b + 1]["out"] + b_out
    return out
